# revision 8
# baseline (speedup 1.0000x reference)
"""Trainium2 Bass kernel for a dense transformer block (pre-LN attention + FFN).

Sharding: 8 cores; core c owns batch b=c//2, query half s=c%2 (1024 tokens).
Each core receives its batch's full sequence feature-major (x^T, fp16) with its
OWN query tokens permuted to columns [0, Tq), so all cores run one SPMD program
(the non-standard zero-diagonal mask lands at compile-time-known tiles).

The wall-clock of a call is dominated by the axon tunnel (~45 MB/s up,
~40 MB/s down), not by on-device execution (<10 ms), so the hot path is
organized around transfer bytes:
  * all weights are cast to fp16 on the host once, shipped to device 0 once,
    broadcast device-to-device, and kept resident across calls;
  * x is shipped fp16 (16-bit matmuls keep PSUM accumulation in fp32, so the
    extra rounding costs ~5e-4 relative error against a 2e-2 gate);
  * the output comes back fp16 and is upcast host-side;
  * the output buffer the NEFF requires as a donated seed is recycled from the
    previous call's device-resident result (nothing shipped).

On-chip layout matches the f32r baseline: activations stay feature-major, LN
statistics via ones-column matmuls, softmax denominator from a ones-column
appended to V. Matmuls that involve weights or activations run fp16x fp16 into
fp32 PSUM; the LN stats/broadcast matmul tricks stay f32r.
"""
import sys

sys.path.insert(0, '/opt/trn_rl_repo')

from contextlib import ExitStack

import numpy as np

import concourse.bass as bass
import concourse.mybir as mybir
import concourse.tile as tile
from concourse.masks import make_identity
from concourse.tile_scheduler import N_PROCS
import bass_rust as _br

F32 = mybir.dt.float32
F32R = mybir.dt.float32r
F16 = mybir.dt.float16
ALU = mybir.AluOpType
ACTF = mybir.ActivationFunctionType

N_CORES = 8
LN_EPS = 1e-5


class ChunkedDrainTileContext(tile.TileContext):
    """walrus's CTRL_NO struct holds very few sync waits; the stock kernel-tail
    drain carries one wait per active semaphore and overflows it. Emit one
    drain per proc instead."""

    def _drain_and_barrier(self, tick_clock, wait_clock):
        g = tick_clock.global_clock
        procs = [i for i in range(N_PROCS) if g.peek_next(i) > 1]
        for p in procs:
            sub = _br.VectorClock()
            sub.require_at_least(p, g.peek_next(p) - 1)
            d = self.nc.sync.drain()
            wait_clock.add_sem_waits(d.ins, _br.ScopedClock({None: sub}))
        self.nc.all_engine_barrier()
        assert self.sems is not None
        popped = self.nc._tile_sem_poison_stack.pop()
        assert popped is self._sem_poison
        self.nc.clear_and_free_semaphores(list(self.sems.allocated().values()))
        self.nc.all_engine_barrier()


def build_program(C=1024, T=2048, Tq=1024, H=16, hs=64, HID=4096, QB=512,
                  split_waits=True):
    """Build the single SPMD per-core program."""
    assert C % 128 == 0 and T % QB == 0 and Tq % QB == 0 and HID % 128 == 0
    assert H % 2 == 0 and H * hs == C and QB % 128 == 0 and hs <= 64
    NCT = C // 128          # feature-dim partition tiles
    NQC = T // QB           # full-sequence column blocks
    NQB = Tq // QB          # owned-query column blocks
    NKT = T // 128          # key-token tiles
    NH1 = HID // 128        # FFN hidden tiles
    KPB = QB // 128         # key tiles overlapping one query block's diagonal
    scale = float(hs) ** -0.5

    nc = bass.Bass(trn_type='TRN2')

    x_tm = nc.declare_dram_parameter("x_tm", [T, C], F16, isOutput=False)
    wq = nc.declare_dram_parameter("wq", [C, C], F16, isOutput=False)
    wk = nc.declare_dram_parameter("wk", [C, C], F16, isOutput=False)
    wv = nc.declare_dram_parameter("wv", [C, C], F16, isOutput=False)
    wo = nc.declare_dram_parameter("wo", [C, C], F16, isOutput=False)
    w1t = nc.declare_dram_parameter("w1t", [NH1, C, 128], F16, isOutput=False)
    w2t = nc.declare_dram_parameter("w2t", [NCT, HID, 128], F16, isOutput=False)
    vec_drams = {}
    for name, n in (("g1", C), ("be1", C), ("g2", C), ("be2", C), ("bo", C),
                    ("b1", HID), ("b2", C)):
        vec_drams[name] = nc.declare_dram_parameter(name, [n], F32, isOutput=False)
    out_tm = nc.declare_dram_parameter("out_tm", [Tq, C], F16, isOutput=True)

    # DRAM staging for K and V(+ones column): lets SBUF hold per-head slices.
    k_stage = nc.dram_tensor("k_stage", [NCT, 128, T], F16)
    v_stage = nc.dram_tensor("v_stage", [NKT, 128, H, hs + 1], F16)

    with ChunkedDrainTileContext(nc) as tc, ExitStack() as top:
        const = top.enter_context(tc.tile_pool(name="const", bufs=1))
        # memset cannot write f32r; stage in f32 and round via DVE copy
        ones32a = const.tile([128, 1], F32)
        nc.vector.memset(ones32a, 1.0)
        ones_col = const.tile([128, 1], F32R)          # lhsT for column sums
        nc.vector.tensor_copy(ones_col, ones32a)
        ones32b = const.tile([1, 128], F32)
        nc.vector.memset(ones32b, 1.0)
        ones_row = const.tile([1, 128], F32R)          # lhsT for broadcasts
        nc.vector.tensor_copy(ones_row, ones32b)
        ones_vst = const.tile([128, H], F16)           # V ones column source
        nc.vector.memset(ones_vst, 1.0)
        ident32 = const.tile([128, 128], F32)
        make_identity(nc, ident32)
        ident16 = const.tile([128, 128], F16)          # PE-transpose operand
        nc.vector.tensor_copy(ident16, ident32)
        dmask = const.tile([128, 128], F32)            # (1 - I)
        nc.vector.tensor_scalar(dmask, ident32, -1.0, 1.0, ALU.mult, ALU.add)
        eps_t = const.tile([1, 1], F32)
        nc.vector.memset(eps_t, LN_EPS)
        eps_col = const.tile([128, 1], F32)
        nc.vector.memset(eps_col, LN_EPS)
        vecs = {}
        for name, dram in vec_drams.items():
            n = dram.shape[0] // 128
            t = const.tile([128, n], F32, tag=f"vec_{name}")
            nc.sync.dma_start(out=t, in_=dram.rearrange("(a p) -> p a", p=128))
            vecs[name] = t

        # Long-lived activation storage with slot reuse across phases:
        #   qx_{ct}: generation 1 = Q (fp16), generation 2 = x2 (fp32)
        #   ah_{ct}: generation 1 = att (fp16), generation 2 = h2 (fp16)
        bigp = top.enter_context(tc.tile_pool(name="bigp", bufs=1))

        def ln_stats(qn, src_of, sps, rows, mu_r, rstd_r, rtag):
            """Column-sum stats via ones-matmuls; writes f32r mu/rstd rows."""
            for qc in range(qn):
                sum_ps = sps.tile([1, QB], F32, tag="sum")
                sq_ps = sps.tile([1, QB], F32, tag="sq")
                for ct in range(NCT):
                    xt = src_of(ct, qc)
                    xtr = rows.tile([128, QB], F32R, tag=rtag + "xr")
                    nc.vector.tensor_copy(xtr, xt)
                    xsq = rows.tile([128, QB], F32R, tag=rtag + "xsq")
                    nc.scalar.activation(xsq, xt, ACTF.Square)
                    nc.tensor.matmul(sum_ps, ones_col, xtr,
                                     start=(ct == 0), stop=(ct == NCT - 1))
                    nc.tensor.matmul(sq_ps, ones_col, xsq,
                                     start=(ct == 0), stop=(ct == NCT - 1))
                mu = rows.tile([1, QB], F32, tag=rtag + "mu")
                nc.vector.tensor_scalar(mu, sum_ps, 1.0 / C, None, ALU.mult)
                ex2 = rows.tile([1, QB], F32, tag=rtag + "ex2")
                nc.vector.tensor_scalar(ex2, sq_ps, 1.0 / C, None, ALU.mult)
                mu2 = rows.tile([1, QB], F32, tag=rtag + "mu2")
                nc.vector.tensor_mul(mu2, mu, mu)
                var = rows.tile([1, QB], F32, tag=rtag + "var")
                nc.vector.tensor_sub(var, ex2, mu2)
                sd = rows.tile([1, QB], F32, tag=rtag + "sd")
                nc.scalar.activation(sd, var, ACTF.Sqrt, bias=eps_t)
                rst = rows.tile([1, QB], F32, tag=rtag + "rst")
                nc.vector.reciprocal(rst, sd)
                nc.vector.tensor_copy(mu_r[:, qc * QB:(qc + 1) * QB], mu)
                nc.vector.tensor_copy(rstd_r[:, qc * QB:(qc + 1) * QB], rst)

        # x feature-major, own tokens only: residual operand for phase 3b.
        xop = top.enter_context(tc.tile_pool(name="xop", bufs=1))
        x_own = [xop.tile([128, Tq], F16, name=f"xo_{ct}", tag=f"xo_{ct}")
                 for ct in range(NCT)]

        # ================= Phase 1+2: LN1, then V/K/Q projections =========
        # Phase 1 works token-major: per-token LN stats via free-dim reduce,
        # normalize with per-partition scalars, then PE-transpose into the
        # feature-major h1 the matmuls want (gamma/beta fold into the
        # PSUM->SBUF eviction).
        with ExitStack() as ph12:
            h1p = ph12.enter_context(tc.tile_pool(name="h1p", bufs=1))
            h1 = [h1p.tile([128, T], F16, name=f"h1_{ct}", tag=f"h1_{ct}") for ct in range(NCT)]

            with ExitStack() as ph1:
                xs = ph1.enter_context(tc.tile_pool(name="xs", bufs=3))
                work = ph1.enter_context(tc.tile_pool(name="wk1", bufs=2))
                st = ph1.enter_context(tc.tile_pool(name="st1", bufs=2))
                tp_ps = ph1.enter_context(tc.tile_pool(name="tp_ps", bufs=4, space="PSUM"))

                for tt in range(T // 128):
                    xt = xs.tile([128, C], F16, tag="x")
                    nc.sync.dma_start(out=xt,
                                      in_=x_tm[tt * 128:(tt + 1) * 128, :])
                    xsq = work.tile([128, C], F32, tag="xsq")
                    nc.scalar.activation(xsq, xt, ACTF.Square)
                    s1 = st.tile([128, 1], F32, tag="s1")
                    nc.vector.tensor_reduce(s1, xt, mybir.AxisListType.X,
                                            ALU.add)
                    s2 = st.tile([128, 1], F32, tag="s2")
                    nc.vector.tensor_reduce(s2, xsq, mybir.AxisListType.X,
                                            ALU.add)
                    mu = st.tile([128, 1], F32, tag="mu")
                    nc.vector.tensor_scalar(mu, s1, 1.0 / C, None, ALU.mult)
                    ex2 = st.tile([128, 1], F32, tag="ex2")
                    nc.vector.tensor_scalar(ex2, s2, 1.0 / C, None, ALU.mult)
                    mu2 = st.tile([128, 1], F32, tag="mu2")
                    nc.vector.tensor_mul(mu2, mu, mu)
                    var = st.tile([128, 1], F32, tag="var")
                    nc.vector.tensor_sub(var, ex2, mu2)
                    sd = st.tile([128, 1], F32, tag="sd")
                    nc.scalar.activation(sd, var, ACTF.Sqrt, bias=eps_col)
                    rstd = st.tile([128, 1], F32, tag="rstd")
                    nc.vector.reciprocal(rstd, sd)
                    xn = xs.tile([128, C], F16, tag="xn")
                    nc.vector.tensor_scalar(xn, xt, mu, rstd,
                                            ALU.subtract, ALU.mult)
                    for ct in range(NCT):
                        pt = tp_ps.tile([128, 128], F16, tag="pt")
                        nc.tensor.transpose(
                            pt, xn[:, ct * 128:(ct + 1) * 128], ident16)
                        nc.vector.tensor_scalar(
                            h1[ct][:, tt * 128:(tt + 1) * 128], pt,
                            vecs["g1"][:, ct:ct + 1], vecs["be1"][:, ct:ct + 1],
                            ALU.mult, ALU.add)
                        if tt < Tq // 128:
                            pr = tp_ps.tile([128, 128], F16, tag="pr")
                            nc.tensor.transpose(
                                pr, xt[:, ct * 128:(ct + 1) * 128], ident16)
                            nc.vector.tensor_copy(
                                x_own[ct][:, tt * 128:(tt + 1) * 128], pr)

            # ---- projections (h1 still resident) ----
            with ExitStack() as ph2:
                wbig = ph2.enter_context(tc.tile_pool(name="wbig", bufs=1))
                ev = ph2.enter_context(tc.tile_pool(name="ev2", bufs=2))
                mps = ph2.enter_context(tc.tile_pool(name="mps", bufs=3, space="PSUM"))
                q_fm = [bigp.tile([128, Tq], F16, name=f"q_{ct}", tag=f"qx_{ct}")
                        for ct in range(NCT)]

                def load_w(dram):
                    out = []
                    for ct in range(NCT):
                        wt = wbig.tile([128, C], F16, tag=f"wr_{ct}")
                        nc.sync.dma_start(out=wt,
                                          in_=dram[ct * 128:(ct + 1) * 128, :])
                        out.append(wt)
                    return out

                # V -> token-major (+ones col), staged
                wv_r = load_w(wv)
                for tmt in range(NKT):
                    vst = ev.tile([128, H, hs + 1], F16, tag="vst")
                    for nb in range(C // QB):
                        ps = mps.tile([128, QB], F32, tag="mm")
                        for ct in range(NCT):
                            nc.tensor.matmul(
                                ps, h1[ct][:, tmt * 128:(tmt + 1) * 128],
                                wv_r[ct][:, nb * QB:(nb + 1) * QB],
                                start=(ct == 0), stop=(ct == NCT - 1))
                        hpb = QB // hs
                        nc.vector.tensor_copy(
                            vst[:, nb * hpb:(nb + 1) * hpb, 0:hs],
                            ps.rearrange("p (h s) -> p h s", s=hs))
                    nc.vector.tensor_copy(
                        vst[:, :, hs:hs + 1],
                        ones_vst.rearrange("p (h o) -> p h o", o=1))
                    nc.sync.dma_start(out=v_stage[tmt], in_=vst)

                # K -> feature-major, staged
                wk_r = load_w(wk)
                for mt in range(NCT):
                    for qc in range(NQC):
                        ps = mps.tile([128, QB], F32, tag="mm")
                        for ct in range(NCT):
                            nc.tensor.matmul(
                                ps, wk_r[ct][:, mt * 128:(mt + 1) * 128],
                                h1[ct][:, qc * QB:(qc + 1) * QB],
                                start=(ct == 0), stop=(ct == NCT - 1))
                        ke = ev.tile([128, QB], F16, tag="ke")
                        nc.vector.tensor_copy(ke, ps)
                        nc.sync.dma_start(
                            out=k_stage[mt][:, qc * QB:(qc + 1) * QB], in_=ke)

                # Q -> feature-major, resident (own tokens only)
                wq_r = load_w(wq)
                for mt in range(NCT):
                    for qc in range(NQB):
                        ps = mps.tile([128, QB], F32, tag="mm")
                        for ct in range(NCT):
                            nc.tensor.matmul(
                                ps, wq_r[ct][:, mt * 128:(mt + 1) * 128],
                                h1[ct][:, qc * QB:(qc + 1) * QB],
                                start=(ct == 0), stop=(ct == NCT - 1))
                        nc.vector.tensor_copy(
                            q_fm[mt][:, qc * QB:(qc + 1) * QB], ps)

        # ================= Phase 3: attention =============================
        att_fm = [bigp.tile([128, Tq], F16, name=f"ah_{ct}", tag=f"ah_{ct}") for ct in range(NCT)]
        with ExitStack() as ph3:
            kv = ph3.enter_context(tc.tile_pool(name="kv", bufs=2))
            epool = ph3.enter_context(tc.tile_pool(name="epool", bufs=4))
            rows3 = ph3.enter_context(tc.tile_pool(name="rows3", bufs=1))
            sc_ps = ph3.enter_context(tc.tile_pool(name="sc_ps", bufs=2, space="PSUM"))
            at_ps = ph3.enter_context(tc.tile_pool(name="at_ps", bufs=1, space="PSUM"))
            br_ps = ph3.enter_context(tc.tile_pool(name="br_ps", bufs=2, space="PSUM"))

            for pair in range(NCT):
                kp = kv.tile([128, T], F16, tag="kp")
                nc.sync.dma_start(out=kp, in_=k_stage[pair])
                vh = []
                for j in range(2):
                    h = 2 * pair + j
                    vraw = kv.tile([128, NKT, hs + 1], F16, tag="vraw")
                    nc.sync.dma_start(
                        out=vraw,
                        in_=v_stage[:, :, h, :].rearrange("kt p s -> p kt s"))
                    vr = kv.tile([128, NKT, hs + 1], F16, tag="vr")
                    nc.scalar.activation(vr, vraw, ACTF.Copy)
                    vh.append(vr)
                for qb in range(NQB):
                    aps = [at_ps.tile([hs + 1, QB], F32, name=f"at{j}", tag=f"at{j}")
                           for j in range(2)]
                    for kt in range(NKT):
                        for j in range(2):
                            sp = sc_ps.tile([128, QB], F32, tag=f"sc{j}")
                            nc.tensor.matmul(
                                sp,
                                kp[j * hs:(j + 1) * hs, kt * 128:(kt + 1) * 128],
                                q_fm[pair][j * hs:(j + 1) * hs,
                                           qb * QB:(qb + 1) * QB],
                                start=True, stop=True)
                            if qb * KPB <= kt < (qb + 1) * KPB:
                                off = (kt - qb * KPB) * 128
                                nc.vector.tensor_mul(
                                    sp[:, off:off + 128],
                                    sp[:, off:off + 128], dmask)
                            et = epool.tile([128, QB], F16, tag="et")
                            nc.scalar.activation(et, sp, ACTF.Exp, scale=scale)
                            nc.tensor.matmul(aps[j], vh[j][:, kt, :], et,
                                             start=(kt == 0),
                                             stop=(kt == NKT - 1))
                    for j in range(2):
                        h = 2 * pair + j
                        rec32 = rows3.tile([1, QB], F32, tag="rec32")
                        nc.vector.reciprocal(rec32, aps[j][hs:hs + 1, :])
                        rec = rows3.tile([1, QB], F32R, tag="rec")
                        nc.vector.tensor_copy(rec, rec32)
                        brc = br_ps.tile([hs, QB], F32, tag="brc")
                        nc.tensor.matmul(brc, ones_row[:, 0:hs], rec,
                                         start=True, stop=True)
                        brc_sb = rows3.tile([hs, QB], F32, tag="brc_sb", bufs=2)
                        nc.vector.tensor_copy(brc_sb, brc)
                        nc.vector.tensor_mul(
                            att_fm[h // 2][(h % 2) * hs:(h % 2) * hs + hs,
                                           qb * QB:(qb + 1) * QB],
                            aps[j][0:hs, :], brc_sb)

        # ================= Phase 3b: output projection + residual =========
        x2 = [bigp.tile([128, Tq], F32, name=f"x2_{ct}", tag=f"qx_{ct}") for ct in range(NCT)]
        with ExitStack() as ph3b:
            wobig = ph3b.enter_context(tc.tile_pool(name="wobig", bufs=1))
            ev3 = ph3b.enter_context(tc.tile_pool(name="ev3", bufs=3))
            op_ps = ph3b.enter_context(tc.tile_pool(name="op_ps", bufs=2, space="PSUM"))
            wo_r = []
            for ct in range(NCT):
                wt = wobig.tile([128, C], F16, tag=f"wo_{ct}")
                nc.sync.dma_start(out=wt, in_=wo[ct * 128:(ct + 1) * 128, :])
                wo_r.append(wt)
            for qb in range(NQB):
                for mt in range(NCT):
                    ps = op_ps.tile([128, QB], F32, tag="ops")
                    for ct in range(NCT):
                        nc.tensor.matmul(
                            ps, wo_r[ct][:, mt * 128:(mt + 1) * 128],
                            att_fm[ct][:, qb * QB:(qb + 1) * QB],
                            start=(ct == 0), stop=(ct == NCT - 1))
                    t1 = ev3.tile([128, QB], F32, tag="sa1")
                    nc.vector.tensor_add(t1, ps,
                                         x_own[mt][:, qb * QB:(qb + 1) * QB])
                    nc.vector.tensor_scalar(
                        x2[mt][:, qb * QB:(qb + 1) * QB], t1,
                        vecs["bo"][:, mt:mt + 1], None, ALU.add)

        # ================= Phase 4: LN2 + FFN + final residual ============
        h2 = [bigp.tile([128, Tq], F16, name=f"ah_{ct}", tag=f"ah_{ct}") for ct in range(NCT)]
        with ExitStack() as ph4a:
            work4 = ph4a.enter_context(tc.tile_pool(name="wk4", bufs=2))
            rows4 = ph4a.enter_context(tc.tile_pool(name="rows4", bufs=1))
            sps4 = ph4a.enter_context(tc.tile_pool(name="sps4", bufs=2, space="PSUM"))
            bps4 = ph4a.enter_context(tc.tile_pool(name="bps4", bufs=2, space="PSUM"))
            mu_r2 = rows4.tile([1, Tq], F32R, tag="mu_r2", bufs=1)
            rstd_r2 = rows4.tile([1, Tq], F32R, tag="rstd_r2", bufs=1)
            ln_stats(NQB, lambda ct, qc: x2[ct][:, qc * QB:(qc + 1) * QB],
                     sps4, work4, mu_r2, rstd_r2, "l2")
            for qc in range(NQB):
                bmu = bps4.tile([128, QB], F32, tag="bmu")
                brs = bps4.tile([128, QB], F32, tag="brs")
                nc.tensor.matmul(bmu, ones_row, mu_r2[:, qc * QB:(qc + 1) * QB],
                                 start=True, stop=True)
                nc.tensor.matmul(brs, ones_row, rstd_r2[:, qc * QB:(qc + 1) * QB],
                                 start=True, stop=True)
                for ct in range(NCT):
                    t1 = work4.tile([128, QB], F32, tag="t1")
                    nc.vector.tensor_sub(t1, x2[ct][:, qc * QB:(qc + 1) * QB], bmu)
                    t2 = work4.tile([128, QB], F32, tag="t2")
                    nc.vector.tensor_mul(t2, t1, brs)
                    nc.vector.tensor_scalar(
                        h2[ct][:, qc * QB:(qc + 1) * QB], t2,
                        vecs["g2"][:, ct:ct + 1], vecs["be2"][:, ct:ct + 1],
                        ALU.mult, ALU.add)

        with ExitStack() as ph4b:
            w1p = ph4b.enter_context(tc.tile_pool(name="w1p", bufs=2))
            hidp = ph4b.enter_context(tc.tile_pool(name="hidp", bufs=1))
            w2p = ph4b.enter_context(tc.tile_pool(name="w2p", bufs=2))
            outp = ph4b.enter_context(tc.tile_pool(name="outp", bufs=2))
            osb = ph4b.enter_context(tc.tile_pool(name="osb", bufs=1))
            f_ps = ph4b.enter_context(tc.tile_pool(name="f_ps", bufs=2, space="PSUM"))
            ot_ps = ph4b.enter_context(tc.tile_pool(name="ot_ps", bufs=4, space="PSUM"))
            W2CH = min(8, NH1)
            TBQ = QB // 128      # token blocks per query block
            for qb in range(NQB):
                out_sb = [osb.tile([128, C], F16, name=f"osb_{tb}",
                                   tag=f"osb_{tb}")
                          for tb in range(TBQ)]
                hid = [hidp.tile([128, QB], F16, name=f"hid_{kt}", tag=f"hid_{kt}")
                       for kt in range(NH1)]
                for kt in range(NH1):
                    w1c = w1p.tile([128, NCT, 128], F16, tag="w1raw")
                    nc.sync.dma_start(
                        out=w1c, in_=w1t[kt].rearrange("(ct p) j -> p ct j", p=128))
                    ps = f_ps.tile([128, QB], F32, tag="h_ps")
                    for ct in range(NCT):
                        nc.tensor.matmul(ps, w1c[:, ct, :],
                                         h2[ct][:, qb * QB:(qb + 1) * QB],
                                         start=(ct == 0), stop=(ct == NCT - 1))
                    nc.scalar.activation(hid[kt], ps, ACTF.Gelu,
                                         bias=vecs["b1"][:, kt:kt + 1])
                for mt in range(NCT):
                    ps = f_ps.tile([128, QB], F32, tag="f_ps")
                    for kc in range(NH1 // W2CH):
                        w2c = w2p.tile([128, W2CH, 128], F16, tag="w2raw")
                        nc.sync.dma_start(
                            out=w2c,
                            in_=w2t[mt][kc * W2CH * 128:(kc + 1) * W2CH * 128]
                            .rearrange("(kt p) j -> p kt j", p=128))
                        for k2 in range(W2CH):
                            kt = kc * W2CH + k2
                            nc.tensor.matmul(ps, w2c[:, k2, :], hid[kt],
                                             start=(kt == 0),
                                             stop=(kt == NH1 - 1))
                    t1 = outp.tile([128, QB], F32, tag="o1")
                    nc.vector.tensor_add(t1, ps,
                                         x2[mt][:, qb * QB:(qb + 1) * QB])
                    t2 = outp.tile([128, QB], F16, tag="o2")
                    nc.vector.tensor_scalar(t2, t1, vecs["b2"][:, mt:mt + 1],
                                            None, ALU.add)
                    for tb in range(TBQ):
                        po = ot_ps.tile([128, 128], F16, tag="po")
                        nc.tensor.transpose(
                            po, t2[:, tb * 128:(tb + 1) * 128], ident16)
                        nc.vector.tensor_copy(
                            out_sb[tb][:, mt * 128:(mt + 1) * 128], po)
                for tb in range(TBQ):
                    nc.sync.dma_start(
                        out=out_tm[(qb * TBQ + tb) * 128:
                                   (qb * TBQ + tb + 1) * 128, :],
                        in_=out_sb[tb])

    if split_waits:
        _split_excess_waits(nc)
    return nc


def _split_excess_waits(nc, max_waits=1):
    """This container's walrus rejects instructions carrying more than ~1-2
    sync waits (per-ISA-struct wait slots). Peel excess waits off onto
    same-engine InstNoOp carriers inserted immediately before the
    instruction — engine queues execute in order, so semantics hold."""
    for f in nc.m.functions:
        for b in f.blocks:
            il = b.instructions  # live list
            out = []
            changed = False
            for inst in il:
                si = inst.sync_info
                if si is not None and len(si.on_wait) > max_waits:
                    waits = list(si.on_wait)
                    extra, keep = waits[:-max_waits], waits[-max_waits:]
                    for k in range(0, len(extra), max_waits):
                        nop = mybir.InstNoOp(name=f"{inst.name}-sw{k}")
                        nop.engine = inst.engine
                        nop.sync_info = mybir.SyncInfo(
                            on_wait=extra[k:k + max_waits], on_update=[])
                        out.append(nop)
                    inst.sync_info = mybir.SyncInfo(
                        on_wait=keep, on_update=list(si.on_update))
                    changed = True
                out.append(inst)
            if changed:
                il[:] = out
    return nc


# ----------------------------------------------------------------------------
# Host-side wrapper: cached compile, device-resident fp16 weights, single
# fp16 x upload per call, recycled donated output seed, fp16 output fetch.
# ----------------------------------------------------------------------------
C_, T_, TQ_, HID_ = 1024, 2048, 1024, 4096
_WNAMES = ("Wq", "Wk", "Wv", "Wo", "bo", "W1", "b1", "W2", "b2",
           "g1", "be1", "g2", "be2")
_cache = {}


def prep_weights(Wq, Wk, Wv, Wo, bo, W1, b1, W2, b2, g1, be1, g2, be2):
    """Host relayout + fp16 cast of everything that is identical across
    cores and across calls."""
    C, HID = C_, HID_
    NCT, NH1 = C // 128, HID // 128
    f16 = np.float16
    wq_c = np.ascontiguousarray(
        np.asarray(Wq, np.float32).transpose(1, 0, 2).reshape(C, C).astype(f16))
    wk_c = np.ascontiguousarray(
        np.asarray(Wk, np.float32).transpose(1, 0, 2).reshape(C, C).astype(f16))
    wv_c = np.ascontiguousarray(
        np.asarray(Wv, np.float32).transpose(1, 0, 2).reshape(C, C).astype(f16))
    wo_c = np.ascontiguousarray(np.asarray(Wo, np.float32).astype(f16))
    w1t = np.ascontiguousarray(
        np.asarray(W1, np.float32).astype(f16).reshape(C, NH1, 128)
        .transpose(1, 0, 2))
    w2t = np.ascontiguousarray(
        np.asarray(W2, np.float32).astype(f16).reshape(HID, NCT, 128)
        .transpose(1, 0, 2))
    return {
        "wq": wq_c, "wk": wk_c, "wv": wv_c, "wo": wo_c, "w1t": w1t, "w2t": w2t,
        "g1": np.asarray(g1, np.float32), "be1": np.asarray(be1, np.float32),
        "g2": np.asarray(g2, np.float32), "be2": np.asarray(be2, np.float32),
        "bo": np.asarray(bo, np.float32), "b1": np.asarray(b1, np.float32),
        "b2": np.asarray(b2, np.float32),
    }


def prep_x(x):
    """Full x [B,T,C] f32 -> global [8*Tq, C] fp16: each core's OWN query
    half only (the partner half is fetched on-device via ppermute, so the
    tunnel carries x exactly once). Pure cast-copies, no transpose."""
    x = np.asarray(x)
    halves = T_ // TQ_
    g = np.empty((N_CORES * TQ_, C_), np.float16)
    for c in range(N_CORES):
        b, s = divmod(c, halves)
        np.copyto(g[c * TQ_:(c + 1) * TQ_], x[b, s * TQ_:(s + 1) * TQ_])
    return g


def assemble_output(host_out, B):
    """Global [8*Tq, C] fp16 (token-major per core) -> [B,T,C] f32."""
    halves = T_ // TQ_
    out = np.empty((B, T_, C_), np.float32)
    for c in range(N_CORES):
        b, s = divmod(c, halves)
        np.copyto(out[b, s * TQ_:(s + 1) * TQ_, :],
                  host_out[c * TQ_:(c + 1) * TQ_, :])
    return out


def _get_exec():
    """Build program + jitted sharded executable once per process."""
    if "exec" in _cache:
        return _cache["exec"]
    import jax
    import jax.numpy as jnp  # noqa: F401  (platform init)
    from jax.sharding import Mesh, PartitionSpec as P, NamedSharding
    try:
        from jax import shard_map
    except ImportError:
        from jax.experimental.shard_map import shard_map
    from concourse.bass2jax import (
        _bass_exec_p, install_neuronx_cc_hook, partition_id_tensor)

    install_neuronx_cc_hook()
    nc = build_program()

    devs = jax.devices()[:N_CORES]
    mesh = Mesh(np.asarray(devs), ("core",))
    sh = NamedSharding(mesh, P("core"))

    partition_name = (nc.partition_id_tensor.name
                      if nc.partition_id_tensor else None)
    in_names, out_names, out_avals = [], [], []
    for alloc in nc.m.functions[0].allocations:
        if not isinstance(alloc, mybir.MemoryLocationSet):
            continue
        name = alloc.memorylocations[0].name
        if alloc.kind == "ExternalInput":
            if name != partition_name:
                in_names.append(name)
        elif alloc.kind == "ExternalOutput":
            out_names.append(name)
            out_avals.append(jax.core.ShapedArray(
                tuple(alloc.tensor_shape), mybir.dt.np(alloc.dtype)))
    n_params = len(in_names)
    all_in_names = in_names + out_names + (
        [partition_name] if partition_name else [])

    def _body(*args):
        operands = list(args)
        if partition_name is not None:
            operands.append(partition_id_tensor())
        return tuple(_bass_exec_p.bind(
            *operands,
            out_avals=tuple(out_avals),
            in_names=tuple(all_in_names),
            out_names=tuple(out_names),
            lowering_input_output_aliases=(),
            sim_require_finite=True,
            sim_require_nnan=True,
            nc=nc,
        ))

    n_outs = len(out_names)
    sharded = jax.jit(
        shard_map(_body, mesh=mesh,
                  in_specs=(P("core"),) * (n_params + n_outs),
                  out_specs=(P("core"),) * n_outs),
        donate_argnums=tuple(range(n_params, n_params + n_outs)),
        keep_unused=True)

    def _exchange(xh):
        # per-core own half [Tq, C] -> [T, C] own-first; partner half comes
        # over the device interconnect instead of the tunnel
        partner = jax.lax.ppermute(
            xh, "core",
            [(0, 1), (1, 0), (2, 3), (3, 2), (4, 5), (5, 4), (6, 7), (7, 6)])
        return jnp.concatenate([xh, partner], axis=0)

    exchange = jax.jit(
        shard_map(_exchange, mesh=mesh, in_specs=P("core"),
                  out_specs=P("core")),
        donate_argnums=(0,))

    def put_replicated(a):
        # one tunnel uplink to dev0, then device-to-device broadcast
        a0 = jax.device_put(np.ascontiguousarray(a), devs[0])
        parts = [a0] + [jax.device_put(a0, d) for d in devs[1:]]
        jax.block_until_ready(parts)
        return jax.make_array_from_single_device_arrays(
            (N_CORES * a.shape[0], *a.shape[1:]), sh, parts)

    ex = {
        "jax": jax, "devs": devs, "sh": sh, "sharded": sharded,
        "exchange": exchange,
        "in_names": in_names, "out_names": out_names, "out_avals": out_avals,
        "put_replicated": put_replicated,
    }
    _cache["exec"] = ex
    return ex


def _weights_device(ex, wdict):
    return {name: ex["put_replicated"](arr) for name, arr in wdict.items()}


def kernel(x, Wq, Wk, Wv, Wo, bo, W1, b1, W2, b2, g1, be1, g2, be2):
    ex = _get_exec()
    jax = ex["jax"]

    win = {"Wq": Wq, "Wk": Wk, "Wv": Wv, "Wo": Wo, "bo": bo, "W1": W1,
           "b1": b1, "W2": W2, "b2": b2, "g1": g1, "be1": be1, "g2": g2,
           "be2": be2}
    wc = _cache.get("weights")
    if wc is not None:
        if not all(win[n] is wc["refs"][n] for n in _WNAMES):
            if all(np.array_equal(np.asarray(win[n]), wc["host"][n])
                   for n in _WNAMES):
                wc["refs"] = dict(win)   # id fast path for later calls
            else:
                wc = None
    if wc is None:
        wdict = prep_weights(**win)
        wc = {
            "refs": dict(win),
            "host": {n: np.asarray(v) for n, v in win.items()},
            "dev": _weights_device(ex, wdict),
        }
        _cache["weights"] = wc

    xg = ex["exchange"](jax.device_put(prep_x(x), ex["sh"]))

    seed = _cache.get("seed")
    if seed is None:
        av = ex["out_avals"][0]
        seed = jax.device_put(
            np.zeros((N_CORES * av.shape[0], *av.shape[1:]), av.dtype),
            ex["sh"])

    args = [xg if n == "x_tm" else wc["dev"][n] for n in ex["in_names"]]
    try:
        outs = ex["sharded"](*args, seed)
    except Exception:
        # the donated seed may have been consumed; rebuild it next call
        _cache.pop("seed", None)
        raise
    _cache["seed"] = outs[0]

    host_out = np.asarray(outs[0])
    return assemble_output(host_out, np.asarray(x).shape[0])
